# revision 2
# baseline (speedup 1.0000x reference)
"""Trainium2 Bass kernel for CustomMultiheadAttention (linear attention with
low-rank QKV projections) — v2: sequence-split sharding + pair AllReduce.

Math (fp32 reference):
    q = elu(query @ q_down_w.T @ q_up_w.T + q_up_b) + 1     # feature map
    k = elu(key   @ k_down_w.T @ k_up_w.T + k_up_b) + 1
    v =      value @ v_down_w.T @ v_up_w.T + v_up_b
    per head h (16 heads, head_dim 64):
        kv_h   = k_h^T v_h                  # [64, 64]  (sum over ALL tokens)
        ksum_h = sum_t k_h[t]               # [64]
        num    = q_h kv_h                   # [S, 64]
        denom  = q_h . ksum_h               # [S]
        attn_h = num / (denom + 1e-6)
    out = concat_h(attn_h) @ out_w.T + out_b

Sharding: 8 cores = 4 batches x 2 sequence-halves. Each core processes 2048
tokens across ALL 16 heads — no duplicated projection work. kv/ksum are sums
over the full sequence, so the two half-cores of a batch AllReduce a packed
[128, 520] f32 buffer (16 diagonal 64x64 kv blocks + ksum) between phase A
(k/v features + partial kv) and phase C (num/denom + output projection).
The AllReduce is hidden behind phase B (q features), which doesn't need kv.

All matmul operands are bf16 (fp32 is not mixable with bf16 on the PE, and
bf16 runs 1 cycle/row at any moving size); PSUM accumulation stays f32.
elu(x)+1 is computed as min(exp(x),1) + max(x,0) so Exp can run straight
from PSUM on the scalar engine. Up-projection biases are folded into the
matmul accumulation via a ones-row stationary (k/v, free-dim bias) or the
activation bias port (q, partition-dim bias). ksum rides along in the kv
einsum as 8 constant-one columns appended to each 256-wide v column group.
"""

import numpy as np

import concourse.bass as bass  # noqa: F401
import concourse.mybir as mybir
import concourse.tile as tile
from concourse import bacc
from concourse.bass_utils import run_bass_kernel_spmd

F32 = mybir.dt.float32
F32R = mybir.dt.float32r
BF16 = mybir.dt.bfloat16
AF = mybir.ActivationFunctionType
OP = mybir.AluOpType

P = 128           # partitions
E = 1024          # embed dim
R = 512           # low rank
T = 2048          # tokens per core (half sequence)
TC = 512          # token chunk
NCHUNK = T // TC  # 4
NE = E // P       # 8 e-tiles
NR = R // P       # 4 r-tiles
NJ = E // P       # 8 j-tiles (head-dim tiles)
NTS = TC // P     # 4 token subtiles per chunk
GW = 264          # v column group width: 256 v cols + 8 ones cols
B, S = 4, 4096

_CACHE = {}


def _build():
    nc = bacc.Bacc(None, target_bir_lowering=False, num_devices=8)

    dp = nc.declare_dram_parameter
    xq = dp("xq", [E, T], BF16, isOutput=False)
    xk = dp("xk", [E, T], BF16, isOutput=False)
    xv = dp("xv", [E, T], BF16, isOutput=False)
    wdq = dp("wdq", [E, R], BF16, isOutput=False)
    wdk = dp("wdk", [E, R], BF16, isOutput=False)
    wdv = dp("wdv", [E, R], BF16, isOutput=False)
    wuq = dp("wuq", [R, E], BF16, isOutput=False)
    wuk = dp("wuk", [R, E], BF16, isOutput=False)
    wuv = dp("wuv", [R, E], BF16, isOutput=False)
    wo = dp("wo", [E, E], BF16, isOutput=False)
    bq = dp("bq", [P, NJ], F32, isOutput=False)     # q bias, [128, j1] layout
    bkb = dp("bkb", [P, E], F32, isOutput=False)    # k bias broadcast
    bvb = dp("bvb", [P, E], F32, isOutput=False)    # v bias broadcast
    r16 = dp("r16", [16, E], BF16, isOutput=False)  # head-replication mask
    rtm = dp("rtm", [P, NJ * 16], BF16, isOutput=False)  # kblk head mask
    out_t = dp("out", [T, E], F32, isOutput=True)

    with tile.TileContext(nc) as tcx:
        from contextlib import ExitStack

        with ExitStack() as root:
            cpool = root.enter_context(tcx.tile_pool(name="consts", bufs=1))
            bq_sb = cpool.tile([P, NJ], F32)
            bkb_sb = cpool.tile([P, E], F32)
            bvb_sb = cpool.tile([P, E], F32)
            r16_sb = cpool.tile([16, E], BF16)
            rtm_sb = cpool.tile([P, NJ * 16], BF16)
            wdq_sb = cpool.tile([P, NE, R], BF16)
            kv_acc = cpool.tile([P, NJ, GW], F32)
            kvs = cpool.tile([P, 520], F32)   # packed diag kv blocks + ksum
            kv2 = cpool.tile([P, NJ, P], BF16)
            kblk = cpool.tile([P, NJ, 16], BF16)
            # two manual vch buffers: ones columns persist across chunks
            vch2 = [cpool.tile([P, NTS, 4, GW], BF16, name=f"vch{i}")
                    for i in range(2)]

            for i in range(2):
                nc.vector.memset(vch2[i][:, :, :, 256:GW], 1.0)
            nc.vector.memset(kv2[:].bitcast(mybir.dt.uint16), 0)

            nc.sync.dma_start(out=bq_sb[:], in_=bq[:])
            nc.sync.dma_start(out=bkb_sb[:], in_=bkb[:])
            nc.sync.dma_start(out=bvb_sb[:], in_=bvb[:])
            nc.sync.dma_start(out=r16_sb[:], in_=r16[:])
            nc.sync.dma_start(out=rtm_sb[:], in_=rtm[:])

            dram = root.enter_context(
                tcx.tile_pool(name="dram", bufs=1, space="DRAM"))
            cc_in = dram.tile([P, 520], F32)
            cc_out = dram.tile([P, 520], F32)

            def fetch_x(x_dram, ci, xpool, tag="x"):
                xt = xpool.tile([P, NE, TC], BF16, tag=tag, name="x")
                nc.sync.dma_start(
                    out=xt[:],
                    in_=x_dram[:, ci * TC:(ci + 1) * TC].rearrange(
                        "(a p) t -> p a t", p=P),
                )
                return xt

            def emit_down(xt, wd_sb, dT, pspool, evac):
                """dT[r, t] (4 r-tiles, bf16) from a fetched x chunk."""
                for rt in range(NR):
                    ps = pspool.tile([P, TC], F32, tag="ps", name="dps")
                    for et in range(NE):
                        nc.tensor.matmul(
                            ps[:], wd_sb[:, et, P * rt:P * (rt + 1)],
                            xt[:, et, :],
                            start=(et == 0), stop=(et == NE - 1),
                        )
                    if evac[rt % len(evac)] == "act":
                        nc.scalar.copy(dT[:, rt, :], ps[:])
                    else:
                        nc.vector.tensor_copy(dT[:, rt, :], ps[:])

            # ---------------- Phase A: k/v features + partial kv ----------
            # wuq/wo live in root scope so their DMA can prefetch in phase A
            wq_sb = cpool.tile([P, NR, E], BF16, name="wq_sb")
            wo_sb = cpool.tile([P, NJ, E], BF16, name="wo_sb")
            xqpool = root.enter_context(tcx.tile_pool(name="xqp", bufs=2))
            with ExitStack() as ph:
                wpool = ph.enter_context(tcx.tile_pool(name="wkv", bufs=1))
                wdk_sb = wpool.tile([P, NE, R], BF16)
                wdv_sb = wpool.tile([P, NE, R], BF16)
                wuk_sb = wpool.tile([P, NR, E], BF16)
                wuv_sb = wpool.tile([P, NR, E], BF16)

                xpool = ph.enter_context(tcx.tile_pool(name="xkv", bufs=3))
                dpool = ph.enter_context(tcx.tile_pool(name="dkv", bufs=2))
                fpool = ph.enter_context(tcx.tile_pool(name="fkv", bufs=2))
                tpool = ph.enter_context(tcx.tile_pool(name="tkv", bufs=4))
                psa = ph.enter_context(
                    tcx.tile_pool(name="psa", bufs=6, space="PSUM"))
                psk = ph.enter_context(
                    tcx.tile_pool(name="psk", bufs=2, space="PSUM"))

                # DMA order: what the PE needs first goes first
                nc.sync.dma_start(
                    out=wdk_sb[:], in_=wdk.rearrange("(a p) r -> p a r", p=P))
                xts = {("k", 0): fetch_x(xk, 0, xpool)}
                nc.sync.dma_start(
                    out=wuk_sb[:], in_=wuk.rearrange("(a p) j -> p a j", p=P))
                nc.sync.dma_start(
                    out=wdv_sb[:], in_=wdv.rearrange("(a p) r -> p a r", p=P))
                xts[("v", 0)] = fetch_x(xv, 0, xpool)
                nc.sync.dma_start(
                    out=wuv_sb[:], in_=wuv.rearrange("(a p) j -> p a j", p=P))
                nc.sync.dma_start(
                    out=wdq_sb[:], in_=wdq.rearrange("(a p) r -> p a r", p=P))

                for ci in range(NCHUNK):
                    dTk = dpool.tile([P, NR, TC], BF16, tag="dTk", name="dTk")
                    dTv = dpool.tile([P, NR, TC], BF16, tag="dTv", name="dTv")
                    kfeat = fpool.tile([P, NTS, E], BF16, tag="kf", name="kf")
                    vch = vch2[ci % 2]

                    xtk = xts.pop(("k", ci))
                    emit_down(xtk, wdk_sb, dTk, psa,
                              ("act", "dve", "act", "dve"))
                    if ci + 1 < NCHUNK:
                        xts[("k", ci + 1)] = fetch_x(xk, ci + 1, xpool)
                    # up k, then feature map on the biased preactivation
                    for ts in range(NTS):
                        for half in range(2):
                            pu = psa.tile([P, TC], F32, tag="ps", name="pu")
                            for rt in range(NR):
                                nc.tensor.matmul(
                                    pu[:], dTk[:, rt, P * ts:P * (ts + 1)],
                                    wuk_sb[:, rt, TC * half:TC * (half + 1)],
                                    start=(rt == 0), stop=(rt == NR - 1),
                                )
                            ub = tpool.tile([P, TC], F32, tag="ub", name="ub")
                            ex = tpool.tile([P, TC], BF16, tag="ex", name="ex")
                            ru = tpool.tile([P, TC], BF16, tag="ru", name="ru")
                            nc.vector.tensor_tensor(
                                ub[:], pu[:],
                                bkb_sb[:, TC * half:TC * (half + 1)],
                                op=OP.add)
                            nc.scalar.activation(ex[:], ub[:], AF.Exp)
                            nc.vector.tensor_scalar_max(ru[:], ub[:], 0.0)
                            nc.vector.scalar_tensor_tensor(
                                kfeat[:, ts, TC * half:TC * (half + 1)],
                                ex[:], 1.0, ru[:], op0=OP.min, op1=OP.add,
                            )

                    xtv = xts.pop(("v", ci))
                    emit_down(xtv, wdv_sb, dTv, psa,
                              ("act", "dve", "act", "dve"))
                    if ci + 1 < NCHUNK:
                        xts[("v", ci + 1)] = fetch_x(xv, ci + 1, xpool)
                    if ci == 0:
                        # prefetch phase-B weights + first q chunk now: the
                        # DMA queue is idle from here to the end of phase A
                        nc.sync.dma_start(
                            out=wq_sb[:],
                            in_=wuq.rearrange("(a p) j -> p a j", p=P))
                        nc.sync.dma_start(
                            out=wo_sb[:],
                            in_=wo.rearrange("(a p) o -> p a o", p=P))
                        xts[("q", 0)] = fetch_x(xq, 0, xqpool, tag="xq")
                    # up v + bias folded into the evacuation add
                    for ts in range(NTS):
                        for half in range(2):
                            pu = psa.tile([P, TC], F32, tag="ps", name="pu")
                            for rt in range(NR):
                                nc.tensor.matmul(
                                    pu[:], dTv[:, rt, P * ts:P * (ts + 1)],
                                    wuv_sb[:, rt, TC * half:TC * (half + 1)],
                                    start=(rt == 0), stop=(rt == NR - 1),
                                )
                            for g in range(2):
                                jj = 2 * half + g
                                nc.vector.tensor_tensor(
                                    vch[:, ts, jj, 0:256],
                                    pu[:, 256 * g:256 * (g + 1)],
                                    bvb_sb[:, 256 * jj:256 * (jj + 1)],
                                    op=OP.add)

                    # kv[j1] += sum_t kfeat[t, j1-dims] vch[t, group(j1)]
                    for j1 in range(NJ):
                        pkv = psk.tile([P, GW], F32, tag="pkv", name="pkv")
                        for ts in range(NTS):
                            nc.tensor.matmul(
                                pkv[:], kfeat[:, ts, P * j1:P * (j1 + 1)],
                                vch[:, ts, j1 // 2, :],
                                start=(ts == 0), stop=(ts == NTS - 1),
                            )
                        if ci == 0:
                            nc.vector.tensor_copy(kv_acc[:, j1, :], pkv[:])
                        else:
                            nc.vector.tensor_tensor(
                                kv_acc[:, j1, :], kv_acc[:, j1, :], pkv[:],
                                op=OP.add)

                # pack diag kv blocks + ksum for the collective
                for j1 in range(NJ):
                    # head h0 = 2*j1 lives at group offset (j1 % 2) * 128
                    off = (j1 % 2) * 128
                    nc.vector.tensor_copy(
                        kvs[0:64, 64 * j1:64 * (j1 + 1)],
                        kv_acc[0:64, j1, off:off + 64])
                    nc.vector.tensor_copy(
                        kvs[64:P, 64 * j1:64 * (j1 + 1)],
                        kv_acc[64:P, j1, off + 64:off + 128])
                    nc.vector.tensor_copy(
                        kvs[:, 512 + j1:513 + j1], kv_acc[:, j1, 256:257])

            # collective on the gpsimd queue so it never blocks sync/DVE
            nc.gpsimd.dma_start(out=cc_in[:], in_=kvs[:])
            nc.gpsimd.collective_compute(
                "AllReduce", OP.add,
                replica_groups=[[0, 1], [2, 3], [4, 5], [6, 7]],
                ins=[cc_in[:].opt()], outs=[cc_out[:].opt()],
            )
            nc.gpsimd.dma_start(out=kvs[:], in_=cc_out[:])

            # ---------------- Phases B (q features) + C (attn + out) ------
            with ExitStack() as ph:
                wuq_sb = wq_sb
                dpool = ph.enter_context(tcx.tile_pool(name="dqp", bufs=2))
                qpool = ph.enter_context(tcx.tile_pool(name="qf", bufs=4))
                tpool = ph.enter_context(tcx.tile_pool(name="tq", bufs=4))
                npool = ph.enter_context(tcx.tile_pool(name="dn", bufs=2))
                apool = ph.enter_context(tcx.tile_pool(name="attn", bufs=2))
                opool = ph.enter_context(tcx.tile_pool(name="osb", bufs=3))
                psb = ph.enter_context(
                    tcx.tile_pool(name="psb", bufs=3, space="PSUM"))
                psdn = ph.enter_context(
                    tcx.tile_pool(name="psdn", bufs=1, space="PSUM"))
                psn = ph.enter_context(
                    tcx.tile_pool(name="psn", bufs=2, space="PSUM"))
                psm = ph.enter_context(
                    tcx.tile_pool(name="psm", bufs=2, space="PSUM"))

                def emit_q(ci):
                    """q features for chunk ci -> qT [j-dims, tokens] bf16."""
                    dTq = dpool.tile([P, NR, TC], BF16, tag="dTq", name="dTq")
                    xtq = xts.pop(("q", ci)) if ("q", ci) in xts else \
                        fetch_x(xq, ci, xqpool, tag="xq")
                    emit_down(xtq, wdq_sb, dTq, psb,
                              ("act", "dve", "act", "dve"))
                    qT = qpool.tile([P, NJ, TC], BF16, tag="qT", name="qT")
                    for j1 in range(NJ):
                        pq = psb.tile([P, TC], F32, tag="ps", name="pq")
                        for rt in range(NR):
                            nc.tensor.matmul(
                                pq[:], wuq_sb[:, rt, P * j1:P * (j1 + 1)],
                                dTq[:, rt, :],
                                start=(rt == 0), stop=(rt == NR - 1),
                            )
                        bias_ap = bq_sb[:, j1:j1 + 1]
                        ex = tpool.tile([P, TC], BF16, tag="qe", name="qe")
                        ru = tpool.tile([P, TC], BF16, tag="qr", name="qr")
                        nc.scalar.activation(ex[:], pq[:], AF.Exp,
                                             bias=bias_ap)
                        nc.vector.tensor_scalar(
                            ru[:], pq[:], bias_ap, 0.0, op0=OP.add, op1=OP.max)
                        nc.vector.scalar_tensor_tensor(
                            qT[:, j1, :], ex[:], 1.0, ru[:],
                            op0=OP.min, op1=OP.add,
                        )
                    return qT

                def emit_denom(qT):
                    """reciprocal of (q . ksum + 1e-6), [16, TC] f32r."""
                    pdn = psdn.tile([16, TC], F32, tag="pdn", name="pdn")
                    for j1 in range(NJ):
                        nc.tensor.matmul(
                            pdn[:], kblk[:, j1, :], qT[:, j1, :],
                            start=(j1 == 0), stop=(j1 == NJ - 1),
                        )
                    dnl = npool.tile([16, TC], F32, tag="dnl", name="dnl")
                    rcpf = npool.tile([16, TC], F32, tag="rcpf", name="rcpf")
                    rcp = npool.tile([16, TC], BF16, tag="rcp", name="rcp")
                    nc.vector.tensor_scalar_add(dnl[:], pdn[:], 1e-6)
                    nc.vector.reciprocal_approx_fast(out=rcpf[:], in_=dnl[:])
                    nc.vector.tensor_copy(rcp[:], rcpf[:])
                    return rcp

                def emit_attn_out(ci, qT, rcp):
                    attn = apool.tile([P, NJ, TC], BF16, tag="at", name="at")
                    for j1 in range(NJ):
                        pnm = psn.tile([P, TC], F32, tag="pnm", name="pnm")
                        nc.tensor.matmul(
                            pnm[:], kv2[:, j1, :], qT[:, j1, :],
                            start=True, stop=True,
                        )
                        prp = psm.tile([P, TC], F32, tag="psm", name="prp")
                        nc.tensor.matmul(
                            prp[:], r16_sb[:, P * j1:P * (j1 + 1)], rcp[:],
                            start=True, stop=True,
                        )
                        rep = tpool.tile([P, TC], F32, tag="rep", name="rep")
                        nc.scalar.copy(rep[:], prp[:])
                        nc.vector.tensor_tensor(
                            attn[:, j1, :], pnm[:], rep[:], op=OP.mult)

                    # out[t, o] = sum_j attn[j, t] wo[j, o]
                    for ts in range(NTS):
                        ob = opool.tile([P, 2, TC], F32, tag="ob", name="ob")
                        for oc in range(2):
                            po = psm.tile([P, TC], F32, tag="psm", name="po")
                            for j1 in range(NJ):
                                nc.tensor.matmul(
                                    po[:], attn[:, j1, P * ts:P * (ts + 1)],
                                    wo_sb[:, j1, TC * oc:TC * (oc + 1)],
                                    start=(j1 == 0), stop=(j1 == NJ - 1),
                                )
                            if oc == 0:
                                nc.scalar.copy(ob[:, oc, :], po[:])
                            else:
                                nc.vector.tensor_copy(ob[:, oc, :], po[:])
                        row0 = ci * TC + ts * P
                        nc.sync.dma_start(
                            out=out_t[row0:row0 + P, :].rearrange(
                                "p (a b) -> p a b", a=2),
                            in_=ob[:],
                        )

                qts = {}
                for ci in range(NCHUNK):
                    qts[ci] = emit_q(ci)

                # kv2 / kblk builds: emitted late so the DVE queue reaches
                # them only after all q features (the collective has the
                # whole phase-B window to complete)
                for j1 in range(NJ):
                    nc.vector.tensor_copy(
                        kv2[0:64, j1, 0:64], kvs[0:64, 64 * j1:64 * (j1 + 1)])
                    nc.vector.tensor_copy(
                        kv2[64:P, j1, 64:P], kvs[64:P, 64 * j1:64 * (j1 + 1)])
                    nc.vector.tensor_scalar(
                        kblk[:, j1, :], rtm_sb[:, 16 * j1:16 * (j1 + 1)],
                        kvs[:, 512 + j1:513 + j1], None, op0=OP.mult)

                for ci in range(NCHUNK):
                    rcp = emit_denom(qts[ci])
                    emit_attn_out(ci, qts[ci], rcp)

    nc.compile()
    return nc


def _get_nc():
    if "nc" not in _CACHE:
        _CACHE["nc"] = _build()
    return _CACHE["nc"]


def kernel(**inputs):
    import ml_dtypes
    bf16 = ml_dtypes.bfloat16

    query = np.asarray(inputs["query"], dtype=np.float32)
    key = np.asarray(inputs["key"], dtype=np.float32)
    value = np.asarray(inputs["value"], dtype=np.float32)

    w = {}
    w["wdq"] = np.ascontiguousarray(inputs["q_down_w"].T).astype(bf16)
    w["wdk"] = np.ascontiguousarray(inputs["k_down_w"].T).astype(bf16)
    w["wdv"] = np.ascontiguousarray(inputs["v_down_w"].T).astype(bf16)
    w["wuq"] = np.ascontiguousarray(inputs["q_up_w"].T).astype(bf16)
    w["wuk"] = np.ascontiguousarray(inputs["k_up_w"].T).astype(bf16)
    w["wuv"] = np.ascontiguousarray(inputs["v_up_w"].T).astype(bf16)
    w["wo"] = np.ascontiguousarray(inputs["out_w"].T).astype(bf16)
    w["bq"] = np.ascontiguousarray(
        np.asarray(inputs["q_up_b"], np.float32).reshape(NJ, P).T)
    w["bkb"] = np.ascontiguousarray(np.broadcast_to(
        np.asarray(inputs["k_up_b"], np.float32), (P, E)))
    w["bvb"] = np.ascontiguousarray(np.broadcast_to(
        np.asarray(inputs["v_up_b"], np.float32), (P, E)))

    heads = np.arange(E) // 64                       # head of each dim
    w["r16"] = np.ascontiguousarray(
        (np.arange(16)[:, None] == heads[None, :]).astype(np.float32)
    ).astype(bf16)
    rtm = np.zeros((P, NJ * 16), np.float32)
    for j1 in range(NJ):
        for p in range(P):
            rtm[p, 16 * j1 + (j1 * P + p) // 64] = 1.0
    w["rtm"] = rtm.astype(bf16)

    in_maps = []
    for c in range(8):
        b, h = divmod(c, 2)
        sl = slice(h * T, (h + 1) * T)
        im = {
            "xq": np.ascontiguousarray(query[b].T[:, sl]).astype(bf16),
            "xk": np.ascontiguousarray(key[b].T[:, sl]).astype(bf16),
            "xv": np.ascontiguousarray(value[b].T[:, sl]).astype(bf16),
        }
        im.update(w)
        in_maps.append(im)

    nc = _get_nc()
    # the first execution after a device wedge occasionally dies with
    # NRT_EXEC_UNIT_UNRECOVERABLE; a retry on a clean session recovers
    last_err = None
    for _attempt in range(3):
        try:
            res = run_bass_kernel_spmd(nc, in_maps, core_ids=list(range(8)),
                                       **_CACHE.get("run_kwargs", {}))
            last_err = None
            break
        except Exception as e:  # noqa: BLE001
            last_err = e
            import time
            time.sleep(10)
    if last_err is not None:
        raise last_err
    _CACHE["last_result"] = res

    out_b = np.asarray(inputs["out_b"], dtype=np.float32)
    out = np.empty((B, S, E), np.float32)
    for c in range(8):
        b, h = divmod(c, 2)
        out[b, h * T:(h + 1) * T] = res.results[c]["out"] + out_b
    return out


# revision 3
# speedup vs baseline: 1.1851x; 1.1851x over previous
"""Trainium2 Bass kernel for CustomMultiheadAttention (linear attention with
low-rank QKV projections) — v5: sequence-split sharding + pair AllReduce.

Math (fp32 reference):
    q = elu(query @ q_down_w.T @ q_up_w.T + q_up_b) + 1     # feature map
    k = elu(key   @ k_down_w.T @ k_up_w.T + k_up_b) + 1
    v =      value @ v_down_w.T @ v_up_w.T (+ v_up_b, folded into out_b)
    per head h (16 heads, head_dim 64):
        kv_h   = k_h^T v_h                  # [64, 64]  (sum over ALL tokens)
        ksum_h = sum_t k_h[t]               # [64]
        denom  = q_h . ksum_h               # [S]
        out_h  = (q_h / denom) kv_h         # [S, 64]
    out = concat_h(out_h) @ out_w.T + out_b'

Sharding: 8 cores = 4 batches x 2 sequence-halves; each core processes 2048
tokens across all 16 heads. kv/ksum are full-sequence sums, so half-core
pairs AllReduce a packed [128, 520] f32 buffer (16 diagonal 64x64 kv blocks
+ ksum); the collective hides behind phase B (q features).

The per-head output is linear in v after the divide, so kv @ out_w collapses
into one per-head matrix M built once after the collective; the output
projection is then (q/denom) @ M — the num einsum and attn intermediate
never materialize.

All matmul operands are bf16 (f32/bf16 cannot mix on the PE; bf16 runs
1 cycle/row at any moving size); PSUM stays f32. elu(x)+1 is computed as
min(exp(x),1) + max(x,0). DVE/ACT instructions carry ~0.5us fixed cost, so
elementwise work is emitted as few wide ops (paired up-projection PSUM
tiles -> [128,1024] ops; multi-dim strided APs for the kv pack/unpack).
"""

import numpy as np

import concourse.bass as bass  # noqa: F401
import concourse.mybir as mybir
import concourse.tile as tile
from concourse import bacc
from concourse.bass_utils import run_bass_kernel_spmd

F32 = mybir.dt.float32
F32R = mybir.dt.float32r
BF16 = mybir.dt.bfloat16
AF = mybir.ActivationFunctionType
OP = mybir.AluOpType

P = 128           # partitions
E = 1024          # embed dim
R = 512           # low rank
T = 2048          # tokens per core (half sequence)
TC = 512          # token chunk
NCHUNK = T // TC  # 4
NE = E // P       # 8 e-tiles
NR = R // P       # 4 r-tiles
NJ = E // P       # 8 j-tiles (head-dim tiles)
NTS = TC // P     # 4 token subtiles per chunk
GW = 136          # per-j1 v column group: 128 v cols + 8 ones cols
B, S = 4, 4096

_CACHE = {}


def _build():
    nc = bacc.Bacc(None, target_bir_lowering=False, num_devices=8)

    dp = nc.declare_dram_parameter
    xq = dp("xq", [E, T], BF16, isOutput=False)
    xk = dp("xk", [E, T], BF16, isOutput=False)
    xv = dp("xv", [E, T], BF16, isOutput=False)
    wdq = dp("wdq", [E, R], BF16, isOutput=False)
    wdk = dp("wdk", [E, R], BF16, isOutput=False)
    wdv = dp("wdv", [E, R], BF16, isOutput=False)
    wuq = dp("wuq", [R, E], BF16, isOutput=False)
    wuk = dp("wuk", [R, E], BF16, isOutput=False)
    wuv = dp("wuv", [R, E], BF16, isOutput=False)
    wo = dp("wo", [E, E], BF16, isOutput=False)
    bq = dp("bq", [P, NJ], F32, isOutput=False)     # q bias, [128, j1] layout
    bkb = dp("bkb", [P, E], F32, isOutput=False)    # k bias broadcast
    idn = dp("idn", [P, P], BF16, isOutput=False)   # identity (PE transpose)
    r16 = dp("r16", [16, E], BF16, isOutput=False)  # head-replication mask
    rtm = dp("rtm", [P, NJ * 16], BF16, isOutput=False)  # kblk head mask
    out_t = dp("out", [T, E], F32, isOutput=True)

    with tile.TileContext(nc) as tcx:
        from contextlib import ExitStack

        with ExitStack() as root:
            cpool = root.enter_context(tcx.tile_pool(name="consts", bufs=1))
            bq_sb = cpool.tile([P, NJ], F32)
            bkb_sb = cpool.tile([P, E], F32)
            idn_sb = cpool.tile([P, P], BF16)
            r16_sb = cpool.tile([16, E], BF16)
            rtm_sb = cpool.tile([P, NJ * 16], BF16)
            wdq_sb = cpool.tile([P, NE, R], BF16)
            wq_sb = cpool.tile([P, NR, E], BF16, name="wq_sb")
            wo_sb = cpool.tile([P, NJ, E], BF16, name="wo_sb")
            m_sb = wo_sb   # M overwrites wo in place after each build matmul
            kv_acc = cpool.tile([P, NJ, GW], F32)
            kvs = cpool.tile([P, 520], F32)   # packed diag kv blocks + ksum
            kv2 = cpool.tile([P, NJ, P], BF16)
            kv2t = cpool.tile([P, NJ, P], BF16)
            kblk = cpool.tile([P, NJ, 16], BF16)
            # two manual vch buffers: ones columns persist across chunks
            vch2 = [cpool.tile([P, NTS, NJ, GW], BF16, name=f"vch{i}")
                    for i in range(2)]

            for i in range(2):
                nc.vector.memset(vch2[i][:, :, :, 128:GW], 1.0)
            nc.vector.memset(kv2[:].bitcast(mybir.dt.uint16), 0)

            nc.scalar.dma_start(out=bq_sb[:], in_=bq[:])
            nc.scalar.dma_start(out=bkb_sb[:], in_=bkb[:])
            nc.scalar.dma_start(out=idn_sb[:], in_=idn[:])
            nc.scalar.dma_start(out=r16_sb[:], in_=r16[:])
            nc.scalar.dma_start(out=rtm_sb[:], in_=rtm[:])

            dram = root.enter_context(
                tcx.tile_pool(name="dram", bufs=1, space="DRAM"))
            cc_in = dram.tile([P, 520], F32)
            cc_out = dram.tile([P, 520], F32)

            xqpool = root.enter_context(tcx.tile_pool(name="xqp", bufs=2))

            def fetch_x(x_dram, ci, xpool, tag="x", split=1, eng=None):
                eng = eng or nc.sync
                xt = xpool.tile([P, NE, TC], BF16, tag=tag, name="x")
                for sp in range(split):
                    a0, a1 = sp * NE // split, (sp + 1) * NE // split
                    eng.dma_start(
                        out=xt[:, a0:a1, :],
                        in_=x_dram[a0 * P:a1 * P,
                                   ci * TC:(ci + 1) * TC].rearrange(
                            "(a p) t -> p a t", p=P),
                    )
                return xt

            def emit_down(xt, wd_sb, dT, pspool, evac):
                """dT[r, t] (4 r-tiles, bf16) from a fetched x chunk."""
                for rt in range(NR):
                    ps = pspool.tile([P, TC], F32, tag="ps", name="dps")
                    for et in range(NE):
                        nc.tensor.matmul(
                            ps[:], wd_sb[:, et, P * rt:P * (rt + 1)],
                            xt[:, et, :],
                            start=(et == 0), stop=(et == NE - 1),
                        )
                    if evac[rt % len(evac)] == "act":
                        nc.scalar.copy(dT[:, rt, :], ps[:])
                    else:
                        nc.vector.tensor_copy(dT[:, rt, :], ps[:])

            # ---------------- Phase A: k/v features + partial kv ----------
            with ExitStack() as ph:
                wpool = ph.enter_context(tcx.tile_pool(name="wkv", bufs=1))
                wdk_sb = wpool.tile([P, NE, R], BF16)
                wdv_sb = wpool.tile([P, NE, R], BF16)
                wuk_sb = wpool.tile([P, NR, E], BF16)
                wuv_sb = wpool.tile([P, NR, E], BF16)

                xpool = ph.enter_context(tcx.tile_pool(name="xkv", bufs=4))
                dpool = ph.enter_context(tcx.tile_pool(name="dkv", bufs=2))
                fpool = ph.enter_context(tcx.tile_pool(name="fkv", bufs=2))
                tpool = ph.enter_context(tcx.tile_pool(name="tkv", bufs=2))
                psd = ph.enter_context(
                    tcx.tile_pool(name="psd", bufs=2, space="PSUM"))
                psu = ph.enter_context(
                    tcx.tile_pool(name="psu", bufs=2, space="PSUM"))
                psk = ph.enter_context(
                    tcx.tile_pool(name="psk", bufs=2, space="PSUM"))

                # DMA order: what the PE needs first goes first, with the
                # first weight/x tiles split so the first matmul can start
                # after only a quarter of the bytes have landed
                for sp in range(4):
                    a0, a1 = sp * NE // 4, (sp + 1) * NE // 4
                    nc.sync.dma_start(
                        out=wdk_sb[:, a0:a1, :],
                        in_=wdk[a0 * P:a1 * P, :].rearrange(
                            "(a p) r -> p a r", p=P))
                xts = {("k", 0): fetch_x(xk, 0, xpool, split=4,
                                          eng=nc.scalar)}
                for sp in range(2):
                    a0, a1 = sp * NR // 2, (sp + 1) * NR // 2
                    nc.sync.dma_start(
                        out=wuk_sb[:, a0:a1, :],
                        in_=wuk[a0 * P:a1 * P, :].rearrange(
                            "(a p) j -> p a j", p=P))
                nc.scalar.dma_start(
                    out=wdv_sb[:], in_=wdv.rearrange("(a p) r -> p a r", p=P))
                xts[("v", 0)] = fetch_x(xv, 0, xpool)
                nc.sync.dma_start(
                    out=wuv_sb[:], in_=wuv.rearrange("(a p) j -> p a j", p=P))
                nc.sync.dma_start(
                    out=wdq_sb[:], in_=wdq.rearrange("(a p) r -> p a r", p=P))

                for ci in range(NCHUNK):
                    dTk = dpool.tile([P, NR, TC], BF16, tag="dTk", name="dTk")
                    dTv = dpool.tile([P, NR, TC], BF16, tag="dTv", name="dTv")
                    kfeat = fpool.tile([P, NTS, E], BF16, tag="kf", name="kf")
                    vch = vch2[ci % 2]

                    xtk = xts.pop(("k", ci))
                    emit_down(xtk, wdk_sb, dTk, psd,
                              ("act", "dve", "act", "dve"))
                    if ci + 1 < NCHUNK:
                        xts[("k", ci + 1)] = fetch_x(xk, ci + 1, xpool)
                    # up k (paired halves -> one [P,1024] psum pair), then
                    # feature map as three wide elementwise ops
                    for ts in range(NTS):
                        pu2 = psu.tile([P, 2, TC], F32, tag="pu", name="pu")
                        for half in range(2):
                            for rt in range(NR):
                                nc.tensor.matmul(
                                    pu2[:, half, :],
                                    dTk[:, rt, P * ts:P * (ts + 1)],
                                    wuk_sb[:, rt, TC * half:TC * (half + 1)],
                                    start=(rt == 0), stop=(rt == NR - 1),
                                )
                        ub = tpool.tile([P, E], F32, tag="ub", name="ub")
                        ex = tpool.tile([P, E], BF16, tag="ex", name="ex")
                        ru = tpool.tile([P, E], BF16, tag="ru", name="ru")
                        nc.vector.tensor_tensor(
                            ub[:], pu2[:].rearrange("p a b -> p (a b)"),
                            bkb_sb[:], op=OP.add)
                        nc.scalar.activation(ex[:], ub[:], AF.Exp)
                        nc.vector.tensor_scalar_max(ru[:], ub[:], 0.0)
                        nc.vector.scalar_tensor_tensor(
                            kfeat[:, ts, :], ex[:], 1.0, ru[:],
                            op0=OP.min, op1=OP.add,
                        )

                    xtv = xts.pop(("v", ci))
                    emit_down(xtv, wdv_sb, dTv, psd,
                              ("act", "dve", "act", "dve"))
                    if ci + 1 < NCHUNK:
                        xts[("v", ci + 1)] = fetch_x(xv, ci + 1, xpool)
                    if ci == 0:
                        # prefetch phase-B weights + first q chunk now: the
                        # DMA queue is idle from here to the end of phase A
                        nc.sync.dma_start(
                            out=wq_sb[:],
                            in_=wuq.rearrange("(a p) j -> p a j", p=P))
                        nc.sync.dma_start(
                            out=wo_sb[:],
                            in_=wo.rearrange("(a p) o -> p a o", p=P))
                        xts[("q", 0)] = fetch_x(xq, 0, xqpool, tag="xq")
                    # up v: one [P,1024] psum pair per ts, one strided copy
                    # into the per-j1 vch groups
                    for ts in range(NTS):
                        pu2 = psu.tile([P, 2, TC], F32, tag="pu", name="pu")
                        for half in range(2):
                            for rt in range(NR):
                                nc.tensor.matmul(
                                    pu2[:, half, :],
                                    dTv[:, rt, P * ts:P * (ts + 1)],
                                    wuv_sb[:, rt, TC * half:TC * (half + 1)],
                                    start=(rt == 0), stop=(rt == NR - 1),
                                )
                        nc.vector.tensor_copy(
                            vch[:, ts, :, 0:P],
                            pu2[:].rearrange("p a (j c) -> p (a j) c", c=P))

                    # kv[j1] += sum_t kfeat[t, j1-dims] vch[t, group(j1)]
                    # paired: two j1 per [P, 2, GW] psum tile (one bank)
                    for jj in range(NJ // 2):
                        pkv2 = psk.tile([P, 2, GW], F32, tag="pkv",
                                        name="pkv")
                        for jh in range(2):
                            j1 = 2 * jj + jh
                            for ts in range(NTS):
                                nc.tensor.matmul(
                                    pkv2[:, jh, :],
                                    kfeat[:, ts, P * j1:P * (j1 + 1)],
                                    vch[:, ts, j1, :],
                                    start=(ts == 0), stop=(ts == NTS - 1),
                                )
                        if ci == 0:
                            nc.vector.tensor_copy(
                                kv_acc[:, 2 * jj:2 * jj + 2, :], pkv2[:])
                        else:
                            nc.vector.tensor_tensor(
                                kv_acc[:, 2 * jj:2 * jj + 2, :],
                                kv_acc[:, 2 * jj:2 * jj + 2, :], pkv2[:],
                                op=OP.add)

                # pack diag kv blocks + ksum for the collective; uniform
                # per-j1 GW stride makes this 3 strided copies
                nc.vector.tensor_copy(
                    kvs[0:64, 0:512].rearrange("p (j c) -> p j c", c=64),
                    kv_acc[0:64, :, 0:64])
                nc.vector.tensor_copy(
                    kvs[64:P, 0:512].rearrange("p (j c) -> p j c", c=64),
                    kv_acc[64:P, :, 64:P])
                nc.vector.tensor_copy(
                    kvs[:, 512:520],
                    kv_acc[:, :, P:P + 1].rearrange("p j c -> p (j c)"))

            # collective on the gpsimd queue so it never blocks sync/DVE
            nc.gpsimd.dma_start(out=cc_in[:], in_=kvs[:])
            nc.gpsimd.collective_compute(
                "AllReduce", OP.add,
                replica_groups=[[0, 1], [2, 3], [4, 5], [6, 7]],
                ins=[cc_in[:].opt()], outs=[cc_out[:].opt()],
            )
            nc.gpsimd.dma_start(out=kvs[:], in_=cc_out[:])

            # ---------------- Phases B (q features) + C (out proj) --------
            with ExitStack() as ph:
                wuq_sb = wq_sb
                dpool = ph.enter_context(tcx.tile_pool(name="dqp", bufs=2))
                qpool = ph.enter_context(tcx.tile_pool(name="qf", bufs=3))
                tpool = ph.enter_context(tcx.tile_pool(name="tq", bufs=4))
                npool = ph.enter_context(tcx.tile_pool(name="dn", bufs=2))
                apool = ph.enter_context(tcx.tile_pool(name="attn", bufs=2))
                opool = ph.enter_context(tcx.tile_pool(name="osb", bufs=3))
                psb = ph.enter_context(
                    tcx.tile_pool(name="psb", bufs=4, space="PSUM"))
                psdn = ph.enter_context(
                    tcx.tile_pool(name="psdn", bufs=1, space="PSUM"))
                psm = ph.enter_context(
                    tcx.tile_pool(name="psm", bufs=2, space="PSUM"))
                pst = ph.enter_context(
                    tcx.tile_pool(name="pst", bufs=1, space="PSUM"))

                def emit_q(ci):
                    """q features for chunk ci -> qT [j-dims, tokens] bf16."""
                    dTq = dpool.tile([P, NR, TC], BF16, tag="dTq", name="dTq")
                    xtq = xts.pop(("q", ci)) if ("q", ci) in xts else \
                        fetch_x(xq, ci, xqpool, tag="xq")
                    # chunk 0 lands while the phase-A ACT tail drains: keep
                    # its psum evacuations off the scalar queue
                    ev = ("dve",) if ci == 0 else ("act", "dve", "act", "dve")
                    emit_down(xtq, wdq_sb, dTq, psb, ev)
                    qT = qpool.tile([P, NJ, TC], BF16, tag="qT", name="qT")
                    for j1 in range(NJ):
                        pq = psb.tile([P, TC], F32, tag="ps", name="pq")
                        for rt in range(NR):
                            nc.tensor.matmul(
                                pq[:], wuq_sb[:, rt, P * j1:P * (j1 + 1)],
                                dTq[:, rt, :],
                                start=(rt == 0), stop=(rt == NR - 1),
                            )
                        bias_ap = bq_sb[:, j1:j1 + 1]
                        ex = tpool.tile([P, TC], BF16, tag="qe", name="qe")
                        ru = tpool.tile([P, TC], BF16, tag="qr", name="qr")
                        nc.scalar.activation(ex[:], pq[:], AF.Exp,
                                             bias=bias_ap)
                        nc.scalar.activation(ru[:], pq[:], AF.Relu,
                                             bias=bias_ap)
                        nc.vector.scalar_tensor_tensor(
                            qT[:, j1, :], ex[:], 1.0, ru[:],
                            op0=OP.min, op1=OP.add,
                        )
                    return qT

                def emit_denom(qT):
                    """reciprocal of (q . ksum + 1e-6), [16, TC] bf16."""
                    pdn = psdn.tile([16, TC], F32, tag="pdn", name="pdn")
                    for j1 in range(NJ):
                        nc.tensor.matmul(
                            pdn[:], kblk[:, j1, :], qT[:, j1, :],
                            start=(j1 == 0), stop=(j1 == NJ - 1),
                        )
                    dnl = npool.tile([16, TC], F32, tag="dnl", name="dnl")
                    rcpf = npool.tile([16, TC], F32, tag="rcpf", name="rcpf")
                    rcp = npool.tile([16, TC], BF16, tag="rcp", name="rcp")
                    nc.vector.tensor_scalar_add(dnl[:], pdn[:], 1e-6)
                    nc.vector.reciprocal_approx_fast(out=rcpf[:], in_=dnl[:])
                    nc.vector.tensor_copy(rcp[:], rcpf[:])
                    return rcp

                def emit_out(ci, qT, rcp):
                    # qs = q / denom (broadcast reciprocal via PE), then
                    # out[t, o] = sum_d qs[d, t] M[d, o]
                    qs = apool.tile([P, NJ, TC], BF16, tag="qs", name="qs")
                    for j1 in range(NJ):
                        prp = psm.tile([P, TC], F32, tag="psm", name="prp")
                        nc.tensor.matmul(
                            prp[:], r16_sb[:, P * j1:P * (j1 + 1)], rcp[:],
                            start=True, stop=True,
                        )
                        nc.vector.tensor_tensor(
                            qs[:, j1, :], qT[:, j1, :], prp[:], op=OP.mult)

                    for ts in range(NTS):
                        ob = opool.tile([P, 2, TC], F32, tag="ob", name="ob")
                        for oc in range(2):
                            po = psm.tile([P, TC], F32, tag="psm", name="po")
                            for j1 in range(NJ):
                                nc.tensor.matmul(
                                    po[:], qs[:, j1, P * ts:P * (ts + 1)],
                                    m_sb[:, j1, TC * oc:TC * (oc + 1)],
                                    start=(j1 == 0), stop=(j1 == NJ - 1),
                                )
                            if oc == 0:
                                nc.scalar.copy(ob[:, oc, :], po[:])
                            else:
                                nc.vector.tensor_copy(ob[:, oc, :], po[:])
                        row0 = ci * TC + ts * P
                        if ci == NCHUNK - 1 and ts == NTS - 1:
                            for oc in range(2):
                                nc.sync.dma_start(
                                    out=out_t[row0:row0 + P,
                                              TC * oc:TC * (oc + 1)],
                                    in_=ob[:, oc, :],
                                )
                        else:
                            nc.sync.dma_start(
                                out=out_t[row0:row0 + P, :].rearrange(
                                    "p (a b) -> p a b", a=2),
                                in_=ob[:],
                            )

                qts = {}
                for ci in range(NCHUNK - 1):
                    qts[ci] = emit_q(ci)

                # post-collective builds. kv2 from the reduced pack (2 wide
                # strided copies on DVE); kblk on the idle Pool engine so the
                # DVE queue stays clear for phase-B evacuations.
                nc.scalar.copy(
                    kv2[0:64, :, 0:64],
                    kvs[0:64, 0:512].rearrange("p (j c) -> p j c", c=64))
                nc.scalar.copy(
                    kv2[64:P, :, 64:P],
                    kvs[64:P, 0:512].rearrange("p (j c) -> p j c", c=64))
                for j1 in range(NJ):
                    nc.gpsimd.tensor_scalar(
                        kblk[:, j1, :], rtm_sb[:, 16 * j1:16 * (j1 + 1)],
                        kvs[:, 512 + j1:513 + j1], None, op0=OP.mult)

                rcps = {0: emit_denom(qts[0])}

                # M[d, o] = sum_e kv[d, e] wo[e, o] per head: transpose the
                # block-diag kv, then one 128-contraction per j1-tile
                for j1 in range(NJ):
                    ptr = pst.tile([P, P], BF16, tag="ptr", name="ptr")
                    nc.tensor.transpose(ptr[:], kv2[:, j1, :], idn_sb[:])
                    if j1 % 2 == 0:
                        nc.scalar.copy(kv2t[:, j1, :], ptr[:])
                    else:
                        nc.vector.tensor_copy(kv2t[:, j1, :], ptr[:])
                for j1 in range(NJ):
                    for oc in range(2):
                        pm = psm.tile([P, TC], F32, tag="psm", name="pm")
                        nc.tensor.matmul(
                            pm[:], kv2t[:, j1, :],
                            wo_sb[:, j1, TC * oc:TC * (oc + 1)],
                            start=True, stop=True,
                        )
                        if oc == 0:
                            nc.scalar.copy(
                                m_sb[:, j1, TC * oc:TC * (oc + 1)], pm[:])
                        else:
                            nc.vector.tensor_copy(
                                m_sb[:, j1, TC * oc:TC * (oc + 1)], pm[:])

                rcps[1] = emit_denom(qts[1])
                emit_out(0, qts[0], rcps[0])
                qts[3] = emit_q(3)
                rcps[2] = emit_denom(qts[2])
                emit_out(1, qts[1], rcps[1])
                rcps[3] = emit_denom(qts[3])
                emit_out(2, qts[2], rcps[2])
                emit_out(3, qts[3], rcps[3])

    nc.compile()
    return nc


def _get_nc():
    if "nc" not in _CACHE:
        _CACHE["nc"] = _build()
    return _CACHE["nc"]


def kernel(**inputs):
    import ml_dtypes
    bf16 = ml_dtypes.bfloat16

    query = np.asarray(inputs["query"], dtype=np.float32)
    key = np.asarray(inputs["key"], dtype=np.float32)
    value = np.asarray(inputs["value"], dtype=np.float32)

    w = {}
    w["wdq"] = np.ascontiguousarray(inputs["q_down_w"].T).astype(bf16)
    w["wdk"] = np.ascontiguousarray(inputs["k_down_w"].T).astype(bf16)
    w["wdv"] = np.ascontiguousarray(inputs["v_down_w"].T).astype(bf16)
    w["wuq"] = np.ascontiguousarray(inputs["q_up_w"].T).astype(bf16)
    w["wuk"] = np.ascontiguousarray(inputs["k_up_w"].T).astype(bf16)
    w["wuv"] = np.ascontiguousarray(inputs["v_up_w"].T).astype(bf16)
    w["wo"] = np.ascontiguousarray(inputs["out_w"].T).astype(bf16)
    w["bq"] = np.ascontiguousarray(
        np.asarray(inputs["q_up_b"], np.float32).reshape(NJ, P).T)
    w["bkb"] = np.ascontiguousarray(np.broadcast_to(
        np.asarray(inputs["k_up_b"], np.float32), (P, E)))
    w["idn"] = np.eye(P, dtype=np.float32).astype(bf16)

    heads = np.arange(E) // 64                       # head of each dim
    w["r16"] = np.ascontiguousarray(
        (np.arange(16)[:, None] == heads[None, :]).astype(np.float32)
    ).astype(bf16)
    rtm = np.zeros((P, NJ * 16), np.float32)
    for j1 in range(NJ):
        for p in range(P):
            rtm[p, 16 * j1 + (j1 * P + p) // 64] = 1.0
    w["rtm"] = rtm.astype(bf16)

    in_maps = []
    for c in range(8):
        b, h = divmod(c, 2)
        sl = slice(h * T, (h + 1) * T)
        im = {
            "xq": np.ascontiguousarray(query[b].T[:, sl]).astype(bf16),
            "xk": np.ascontiguousarray(key[b].T[:, sl]).astype(bf16),
            "xv": np.ascontiguousarray(value[b].T[:, sl]).astype(bf16),
        }
        im.update(w)
        in_maps.append(im)

    nc = _get_nc()
    # the first execution after a device wedge occasionally dies with
    # NRT_EXEC_UNIT_UNRECOVERABLE; a retry on a clean session recovers
    last_err = None
    for _attempt in range(3):
        try:
            res = run_bass_kernel_spmd(nc, in_maps, core_ids=list(range(8)),
                                       **_CACHE.get("run_kwargs", {}))
            last_err = None
            break
        except Exception as e:  # noqa: BLE001
            last_err = e
            import time
            time.sleep(10)
    if last_err is not None:
        raise last_err
    _CACHE["last_result"] = res

    # v_up bias contributes a constant row: (num_bv/denom) == v_up_b
    out_b = np.asarray(inputs["out_b"], dtype=np.float32) + \
        np.asarray(inputs["v_up_b"], np.float32) @ \
        np.asarray(inputs["out_w"], np.float32).T
    out = np.empty((B, S, E), np.float32)
    for c in range(8):
        b, h = divmod(c, 2)
        out[b, h * T:(h + 1) * T] = res.results[c]["out"] + out_b
    return out
